# revision 11
# baseline (speedup 1.0000x reference)
"""Trainium2 Bass kernel for MeanAggSelfAttentionBlock (B=16,N=1024,D=512,H=8,L=4).

Strategy: pure data-parallel over batch (2 batches/core on 8 cores, no
collectives).  Activations are kept feature-major (h^T: [D, N]) in SBUF so
every linear layer is a natural lhsT(=W) x rhs(=h^T) matmul.  Attention is
computed fully transposed (S^T = K^T.T @ Q^T per head, softmax over the
partition axis) with the softmax denominator folded into the A@V matmul as an
appended ones-column on V.  LayerNorm statistics are computed with
ones-matmuls whose M=128 output replicates mean across all partitions, so no
partition broadcasts are needed for the normalize.  LN affine params and the
q-scale are folded into the projection weights on the host; matmuls run in
fp32r (full PE speed at >=256 free dim); the attention inner path and the
MLP's second matmul run in bf16.
"""

import numpy as np
import ml_dtypes

import concourse.bass as bass
import concourse.tile as tile
from concourse import mybir, bacc
from concourse.bass_utils import run_bass_kernel_spmd

B, N, D, H, L, WF = 16, 1024, 512, 8, 4, 4
HD, FF = D // H, WF * D
NCORES = 8
BL = B // NCORES          # batches per core
EPS = 1e-5
ROPE_BASE = 10000.0

DT = D // 128             # 4   feature tiles
NT = N // 128             # 8   token tiles
FT = FF // 128            # 16  mlp hidden tiles
ISL = N // 512            # 2   512-wide free-dim slices

F32 = mybir.dt.float32
F32R = mybir.dt.float32r
BF16 = mybir.dt.bfloat16
AF = mybir.ActivationFunctionType
OP = mybir.AluOpType
AX = mybir.AxisListType


def _r(ap):
    return ap.bitcast(F32R)


def _ds(i, sz=512):
    return bass.ds(i * sz, sz)


# ---------------------------------------------------------------------------
# device program
# ---------------------------------------------------------------------------

def build_program(flags):
    has_bv, has_bo, has_b2 = flags
    nc = bacc.Bacc("TRN2", target_bir_lowering=False, debug=False,
                   num_devices=NCORES)

    t = {}
    def din(name, shape, dt=F32):
        t[name] = nc.dram_tensor(name, list(shape), dt, kind="ExternalInput").ap()

    din("xT", (BL, D, N), F32R)
    din("wq", (L, D, D), F32R); din("wk", (L, D, D), F32R); din("wv", (L, D, D), F32R)
    din("wo", (L, D, D), F32R)
    din("w1", (L, D, FF), F32R); din("w2", (L, FF, D), BF16)
    din("pbias", (128, L * 24))          # per layer: bq[4] bk[4] b1[16]
    din("cosT", (128, N)); din("sinT", (128, N)); din("p128", (128, 128), F32R)
    din("maskbias", (128, BL * NT))      # exp bias per (batch, jt): [j-in-tile]
    din("keepn", (BL, N))                # pooling weights (1/count kept)
    din("ones_stat", (128, 128), F32R)
    din("ones64", (1, 64), F32R)
    if has_bv:
        din("bv", (L, D))
    if has_bo:
        din("bo", (L, D))
    if has_b2:
        din("b2", (L, D))
    out = nc.dram_tensor("out", [BL, D], F32, kind="ExternalOutput").ap()

    from contextlib import ExitStack
    with tile.TileContext(nc) as tc:
        with ExitStack() as ctx:
            _emit(ctx, tc, t, out, flags)
    nc.compile()
    return nc


def _emit(ctx, tc, t, out, flags):
    has_bv, has_bo, has_b2 = flags
    nc = tc.nc

    const = ctx.enter_context(tc.tile_pool(name="const", bufs=1))
    wpool = ctx.enter_context(tc.tile_pool(name="w", bufs=1))
    hpool = ctx.enter_context(tc.tile_pool(name="h", bufs=BL * DT))
    s1024 = ctx.enter_context(tc.tile_pool(name="s1024", bufs=5))
    qkp = ctx.enter_context(tc.tile_pool(name="qk", bufs=DT))
    vp = ctx.enter_context(tc.tile_pool(name="v", bufs=NT))
    esp = ctx.enter_context(tc.tile_pool(name="es", bufs=2))
    up = ctx.enter_context(tc.tile_pool(name="u", bufs=4))
    t512 = ctx.enter_context(tc.tile_pool(name="t512", bufs=6))
    small = ctx.enter_context(tc.tile_pool(name="small", bufs=2))
    psum = ctx.enter_context(tc.tile_pool(name="psum", bufs=8, space="PSUM"))

    def ps_tile():
        return psum.tile([128, 512], F32, tag="ps", name="ps")

    # ---- constants -------------------------------------------------------
    ones_stat = const.tile([128, 128], F32R)
    nc.sync.dma_start(out=ones_stat[:], in_=t["ones_stat"][:])
    eps_t = const.tile([128, 1], F32)
    nc.vector.memset(eps_t[:], EPS)
    ones64 = const.tile([1, 64], F32R)
    nc.sync.dma_start(out=ones64[:], in_=t["ones64"][:])
    cosT = const.tile([128, N], F32)
    nc.sync.dma_start(out=cosT[:], in_=t["cosT"][:])
    sinT = const.tile([128, N], F32)
    nc.sync.dma_start(out=sinT[:], in_=t["sinT"][:])
    p128 = const.tile([128, 128], F32R)
    nc.sync.dma_start(out=p128[:], in_=t["p128"][:])
    pbias = const.tile([128, L * 24], F32)
    nc.sync.dma_start(out=pbias[:], in_=t["pbias"][:])
    maskbias = const.tile([128, BL * NT], F32)
    nc.sync.dma_start(out=maskbias[:], in_=t["maskbias"][:])
    if has_bo:
        ones1 = const.tile([1, 512], F32)
        nc.vector.memset(ones1[:], 1.0)
    if has_b2:
        ones1b = const.tile([1, 512], BF16)
        nc.vector.memset(ones1b[:], 1.0)

    # ---- load x (feature-major) -----------------------------------------
    h_t = {}
    for b in range(BL):
        h_t[b] = []
        for dt in range(DT):
            ht = hpool.tile([128, N], F32R, tag="h", name="h")
            nc.sync.dma_start(out=ht[:], in_=t["xT"][b, dt * 128:(dt + 1) * 128, :])
            h_t[b].append(ht)

    # ---- layernorm emitter ----------------------------------------------
    def emit_ln(h_tiles, y_tiles):
        for isl in range(ISL):
            s = _ds(isl)
            ps_mu = ps_tile()
            for dt in range(DT):
                nc.tensor.matmul(ps_mu[:], _r(ones_stat[:]), _r(h_tiles[dt][:, s]),
                                 start=(dt == 0), stop=(dt == DT - 1))
            ps_m2 = ps_tile()
            for dt in range(DT):
                sq = t512.tile([128, 512], F32R, tag="t512", name="t512")
                nc.vector.tensor_mul(sq[:], h_tiles[dt][:, s], h_tiles[dt][:, s])
                nc.tensor.matmul(ps_m2[:], _r(ones_stat[:]), _r(sq[:]),
                                 start=(dt == 0), stop=(dt == DT - 1))
            musq = t512.tile([128, 512], F32R, tag="t512", name="t512")
            nc.scalar.activation(musq[:], ps_mu[:], AF.Square)
            var = t512.tile([128, 512], F32, tag="t512", name="t512")
            nc.vector.tensor_tensor(var[:], ps_m2[:], musq[:], OP.subtract)
            std = t512.tile([128, 512], F32, tag="t512", name="t512")
            nc.scalar.activation(std[:], var[:], AF.Sqrt, bias=eps_t[:])
            rstd = t512.tile([128, 512], F32, tag="rstd", name="rstd", bufs=2)
            nc.vector.reciprocal(rstd[:], std[:])
            for dt in range(DT):
                tmp = t512.tile([128, 512], F32, tag="t512", name="t512")
                nc.vector.tensor_tensor(tmp[:], h_tiles[dt][:, s], ps_mu[:],
                                        OP.subtract)
                nc.vector.tensor_tensor(y_tiles[dt][:, s], tmp[:], rstd[:],
                                        OP.mult)

    # ---- per layer -------------------------------------------------------
    for l in range(L):
        wq_sb = wpool.tile([128, DT, D], F32R, tag="wq")
        nc.sync.dma_start(out=wq_sb[:], in_=t["wq"][l].rearrange("(dt p) c -> p dt c", p=128))
        wk_sb = wpool.tile([128, DT, D], F32R, tag="wk")
        nc.sync.dma_start(out=wk_sb[:], in_=t["wk"][l].rearrange("(dt p) c -> p dt c", p=128))
        wv_sb = wpool.tile([128, DT, D], F32R, tag="wv")
        nc.sync.dma_start(out=wv_sb[:], in_=t["wv"][l].rearrange("(dt p) c -> p dt c", p=128))
        wo_sb = wpool.tile([128, DT, D], F32R, tag="wo")
        nc.sync.dma_start(out=wo_sb[:], in_=t["wo"][l].rearrange("(ct p) f -> p ct f", p=128))
        w1_sb = wpool.tile([128, DT, FF], F32R, tag="w1")
        nc.sync.dma_start(out=w1_sb[:], in_=t["w1"][l].rearrange("(dt p) f -> p dt f", p=128))
        w2_sb = wpool.tile([128, FT, D], BF16, tag="w2")
        nc.sync.dma_start(out=w2_sb[:], in_=t["w2"][l].rearrange("(ft p) c -> p ft c", p=128))
        if has_bv:
            bv_rep = wpool.tile([128, D], F32, tag="bvr")
            src = t["bv"][l]
            nc.sync.dma_start(out=bv_rep[:], in_=bass.AP(
                tensor=src.tensor, offset=src.offset, ap=[[0, 128]] + src.ap))
        if has_bo:
            bo_row = wpool.tile([1, D], F32, tag="bor")
            nc.sync.dma_start(out=bo_row[:], in_=t["bo"][l][None, :])
        if has_b2:
            b2_row = wpool.tile([1, D], BF16, tag="b2r")
            nc.sync.dma_start(out=b2_row[:], in_=t["b2"][l][None, :])

        for b in range(BL):
            # ---------- LN1 ----------
            y_t = [s1024.tile([128, N], F32R, tag="s1024", name="s1024") for _ in range(DT)]
            emit_ln(h_t[b], y_t)

            # ---------- Q^T, K^T projections ----------
            qt_t, kt_t = [], []
            for which, w_sb, bcol, lst in (("q", wq_sb, 0, qt_t),
                                           ("k", wk_sb, 4, kt_t)):
                for ct in range(DT):
                    dst = qkp.tile([128, N], BF16, tag=which + "t", name=which + "t")
                    for isl in range(ISL):
                        s = _ds(isl)
                        ps = ps_tile()
                        for dt in range(DT):
                            nc.tensor.matmul(
                                ps[:], _r(w_sb[:, dt, ct * 128:(ct + 1) * 128]),
                                _r(y_t[dt][:, s]),
                                start=(dt == 0), stop=(dt == DT - 1))
                        bias_ap = pbias[:, l * 24 + bcol + ct:l * 24 + bcol + ct + 1]
                        if l == 0:
                            raw = t512.tile([128, 512], F32R, tag="t512", name="t512")
                            nc.scalar.activation(raw[:], ps[:], AF.Identity,
                                                 bias=bias_ap)
                            ps_rot = ps_tile()
                            nc.tensor.matmul(ps_rot[:], _r(p128[:]), _r(raw[:]),
                                             start=True, stop=True)
                            t1 = t512.tile([128, 512], F32, tag="t512", name="t512")
                            nc.vector.tensor_tensor(t1[:], raw[:], cosT[:, s],
                                                    OP.mult)
                            t2 = t512.tile([128, 512], F32, tag="t512", name="t512")
                            nc.vector.tensor_tensor(t2[:], ps_rot[:], sinT[:, s],
                                                    OP.mult)
                            nc.vector.tensor_tensor(dst[:, s], t1[:], t2[:],
                                                    OP.add)
                        else:
                            nc.scalar.activation(dst[:, s], ps[:], AF.Identity,
                                                 bias=bias_ap)
                    lst.append(dst)

            # ---------- V projection (position-major, +ones column) ----------
            v_t = []
            for jt in range(NT):
                ps = ps_tile()
                for dt in range(DT):
                    nc.tensor.matmul(ps[:], _r(y_t[dt][:, jt * 128:(jt + 1) * 128]),
                                     _r(wv_sb[:, dt, :]),
                                     start=(dt == 0), stop=(dt == DT - 1))
                v65 = vp.tile([128, H * 65], BF16, tag="v65", name="v65")
                vv = v65[:].rearrange("p (g m) -> p g m", m=65)
                pv = ps[:].rearrange("p (g c) -> p g c", c=64)
                if has_bv:
                    nc.vector.tensor_tensor(
                        vv[:, :, 0:64], pv,
                        bv_rep[:].rearrange("p (g c) -> p g c", c=64), OP.add)
                else:
                    nc.vector.tensor_copy(vv[:, :, 0:64], pv)
                nc.vector.memset(vv[:, :, 64], 1.0)
                v_t.append(v65)

            # ---------- attention, head-pair at a time ----------
            ot_t = []
            for hp in range(DT):
                ot = s1024.tile([128, N], F32R, tag="s1024", name="s1024")
                ps_o = [[psum.tile([128, 512], F32, tag="ps", name="ps")
                         for _ in range(ISL)] for _ in range(2)]
                for jt in range(NT):
                    es_t = [esp.tile([128, N], BF16, tag=f"es{hh}", name=f"es{hh}")
                            for hh in range(2)]
                    for isl in range(ISL):
                        s = _ds(isl)
                        ps_s = []
                        for hh in range(2):
                            pb = hh * 64
                            p_ = ps_tile()
                            nc.tensor.matmul(
                                p_[:], kt_t[hp][pb:pb + 64, jt * 128:(jt + 1) * 128],
                                qt_t[hp][pb:pb + 64, s], start=True, stop=True)
                            ps_s.append(p_)
                        mb_ap = maskbias[:, b * NT + jt:b * NT + jt + 1]
                        for hh in range(2):
                            nc.scalar.activation(es_t[hh][:, s], ps_s[hh][:],
                                                 AF.Exp, bias=mb_ap)
                    for hh in range(2):
                        g = hp * 2 + hh
                        lhs = v_t[jt][:].rearrange("p (g m) -> p g m", m=65)[:, g, :]
                        for isl in range(ISL):
                            nc.tensor.matmul(
                                ps_o[hh][isl][0:65, :], lhs,
                                es_t[hh][:, _ds(isl)],
                                start=(jt == 0), stop=(jt == NT - 1))
                # normalize + evict
                for hh in range(2):
                    for isl in range(ISL):
                        rd = small.tile([1, 512], F32R, tag="rd", name="rd")
                        with nc.allow_low_precision(reason="fp32r denom"):
                            nc.vector.reciprocal(rd[:], ps_o[hh][isl][64:65, :])
                        ps_rep = ps_tile()
                        nc.tensor.matmul(ps_rep[0:64, :], _r(ones64[:]),
                                         _r(rd[:]), start=True, stop=True)
                        rdrep = small.tile([64, 512], F32, tag="rdrep", name="rdrep")
                        nc.vector.tensor_copy(rdrep[:], ps_rep[0:64, :])
                        nc.vector.tensor_tensor(
                            ot[hh * 64:(hh + 1) * 64, _ds(isl)],
                            ps_o[hh][isl][0:64, :], rdrep[:], OP.mult)
                ot_t.append(ot)

            # ---------- output projection + residual ----------
            for ft in range(DT):
                for isl in range(ISL):
                    s = _ds(isl)
                    ps = ps_tile()
                    for ct in range(DT):
                        nc.tensor.matmul(
                            ps[:], _r(wo_sb[:, ct, ft * 128:(ft + 1) * 128]),
                            _r(ot_t[ct][:, s]),
                            start=(ct == 0),
                            stop=(ct == DT - 1 and not has_bo))
                    if has_bo:
                        nc.tensor.matmul(ps[:],
                                         _r(bo_row[0:1, ft * 128:(ft + 1) * 128]),
                                         _r(ones1[:]), start=False, stop=True)
                    nc.vector.tensor_tensor(h_t[b][ft][:, s], ps[:],
                                            h_t[b][ft][:, s], OP.add)

            # ---------- LN2 + MLP ----------
            m_t = [s1024.tile([128, N], F32R, tag="s1024", name="s1024") for _ in range(DT)]
            emit_ln(h_t[b], m_t)
            for isl in range(ISL):
                s = _ds(isl)
                ps_m2 = [ps_tile() for _ in range(DT)]
                for ft in range(FT):
                    ps_u = ps_tile()
                    for dt in range(DT):
                        nc.tensor.matmul(
                            ps_u[:], _r(w1_sb[:, dt, ft * 128:(ft + 1) * 128]),
                            _r(m_t[dt][:, s]),
                            start=(dt == 0), stop=(dt == DT - 1))
                    u = up.tile([128, 512], BF16, tag="u", name="u")
                    bcol = l * 24 + 8 + ft
                    nc.scalar.activation(u[:], ps_u[:], AF.Gelu,
                                         bias=pbias[:, bcol:bcol + 1])
                    for dt in range(DT):
                        nc.tensor.matmul(
                            ps_m2[dt][:], w2_sb[:, ft, dt * 128:(dt + 1) * 128],
                            u[:], start=(ft == 0),
                            stop=(ft == FT - 1 and not has_b2))
                for dt in range(DT):
                    if has_b2:
                        nc.tensor.matmul(ps_m2[dt][:],
                                         b2_row[0:1, dt * 128:(dt + 1) * 128],
                                         ones1b[:], start=False, stop=True)
                    nc.vector.tensor_tensor(h_t[b][dt][:, s], ps_m2[dt][:],
                                            h_t[b][dt][:, s], OP.add)

    # ---- mean pooling ----------------------------------------------------
    outv = out.rearrange("b (dt p) -> b p dt", p=128)
    for b in range(BL):
        src = t["keepn"][b]
        keeprep = small.tile([128, N], F32, tag="keeprep", name="keeprep", bufs=1)
        nc.sync.dma_start(out=keeprep[:], in_=bass.AP(
            tensor=src.tensor, offset=src.offset, ap=[[0, 128]] + src.ap))
        pooled = small.tile([128, DT], F32, tag="pooled", name="pooled")
        for dt in range(DT):
            pm = s1024.tile([128, N], F32R, tag="s1024", name="s1024")
            nc.vector.tensor_tensor(pm[:], h_t[b][dt][:], keeprep[:], OP.mult)
            nc.vector.reduce_sum(pooled[:, dt:dt + 1], pm[:], axis=AX.X)
        nc.sync.dma_start(out=outv[b], in_=pooled[:])


# ---------------------------------------------------------------------------
# host side
# ---------------------------------------------------------------------------

_PROGRAMS = {}


def _host_prep(inputs):
    f32 = np.float32
    g = lambda k: np.asarray(inputs[k], f32)
    scale = HD ** -0.5
    ln1w, ln1b = g("ln1_w"), g("ln1_b")
    ln2w, ln2b = g("ln2_w"), g("ln2_b")
    wq, bq = g("wq"), g("bq")
    wk, bk = g("wk"), g("bk")
    wv, bv = g("wv"), g("bv")
    w1, b1 = g("w1"), g("b1")
    d = {}
    d["wq"] = np.ascontiguousarray(ln1w[:, :, None] * wq * scale)
    d["wk"] = np.ascontiguousarray(ln1w[:, :, None] * wk)
    d["wv"] = np.ascontiguousarray(ln1w[:, :, None] * wv)
    d["wo"] = np.ascontiguousarray(g("wo"))
    d["w1"] = np.ascontiguousarray(ln2w[:, :, None] * w1)
    d["w2"] = np.ascontiguousarray(g("w2")).astype(ml_dtypes.bfloat16)
    bq_e = (bq + np.einsum("ld,ldf->lf", ln1b, wq)) * scale     # [L, D]
    bk_e = bk + np.einsum("ld,ldf->lf", ln1b, wk)
    bv_e = bv + np.einsum("ld,ldf->lf", ln1b, wv)
    b1_e = b1 + np.einsum("ld,ldf->lf", ln2b, w1)               # [L, FF]
    # pbias [128, L*24]: per layer bq[4] bk[4] b1[16], col k = tile k
    pb = np.zeros((128, L * 24), f32)
    for l in range(L):
        pb[:, l * 24 + 0:l * 24 + 4] = bq_e[l].reshape(4, 128).T
        pb[:, l * 24 + 4:l * 24 + 8] = bk_e[l].reshape(4, 128).T
        pb[:, l * 24 + 8:l * 24 + 24] = b1_e[l].reshape(16, 128).T
    d["pbias"] = pb
    # rope tables (feature-major, two heads stacked)
    inv_freq = 1.0 / (ROPE_BASE ** (np.arange(0, HD, 2, dtype=f32) / HD))
    freqs = np.arange(N, dtype=f32)[:, None] * inv_freq[None, :]
    freqs = np.repeat(freqs, 2, axis=-1)                        # [N, HD]
    cos64, sin64 = np.cos(freqs).T, np.sin(freqs).T             # [HD, N]
    d["cosT"] = np.ascontiguousarray(np.concatenate([cos64, cos64], 0), dtype=f32)
    d["sinT"] = np.ascontiguousarray(np.concatenate([sin64, sin64], 0), dtype=f32)
    M = np.zeros((HD, HD), f32)
    for k in range(HD // 2):
        M[2 * k, 2 * k + 1] = -1.0
        M[2 * k + 1, 2 * k] = 1.0
    P = np.zeros((128, 128), f32)
    P[:HD, :HD] = M
    P[HD:, HD:] = M
    d["p128"] = P
    d["ones_stat"] = np.full((128, 128), 1.0 / D, f32)
    d["ones64"] = np.ones((1, 64), f32)
    flags = (bool(np.any(bv_e)), bool(np.any(g("bo"))), bool(np.any(g("b2"))))
    if flags[0]:
        d["bv"] = bv_e
    if flags[1]:
        d["bo"] = g("bo")
    if flags[2]:
        d["b2"] = g("b2").astype(ml_dtypes.bfloat16)
    return d, flags


def _per_core_inputs(inputs, common):
    x = np.asarray(inputs["x"], np.float32)
    pad = np.asarray(inputs["pad_mask"])
    maps = []
    for c in range(NCORES):
        m = dict(common)
        xs = x[c * BL:(c + 1) * BL]                       # [BL, N, D]
        m["xT"] = np.ascontiguousarray(xs.transpose(0, 2, 1))
        mb = np.zeros((128, BL * NT), np.float32)
        kp = np.zeros((BL, N), np.float32)
        for b in range(BL):
            pm = pad[c * BL + b]                          # [N] bool
            bias = np.where(pm, np.float32(-1e30), np.float32(0.0))
            mb[:, b * NT:(b + 1) * NT] = bias.reshape(NT, 128).T
            keep = (~pm).astype(np.float32)
            kp[b] = keep / max(keep.sum(), 1.0)
        m["maskbias"] = mb
        m["keepn"] = kp
        maps.append(m)
    return maps


def kernel(**inputs):
    common, flags = _host_prep(inputs)
    nc = _PROGRAMS.get(flags)
    if nc is None:
        nc = build_program(flags)
        _PROGRAMS[flags] = nc
    in_maps = _per_core_inputs(inputs, common)
    res = run_bass_kernel_spmd(nc, in_maps, core_ids=list(range(NCORES)))
    outs = [res.results[c]["out"] for c in range(NCORES)]
    return np.concatenate(outs, axis=0).astype(np.float32)
